# revision 1
# baseline (speedup 1.0000x reference)
"""Trainium2 Bass kernel for nn_Attention_44504451121208.

Dual-stream (x / x_hsi) 12-head attention block:
  qkv -> [template-template attn (shared), search-all attn per stream] -> proj.

Strategy: data-parallel over batch B=64 across 8 NeuronCores (8 batches/core,
no collectives). All matmuls in bf16 (fp32 accumulation in PSUM).

Layout notes (per core):
 - Tokens are reordered internally to [search 256 | template 128] so the
   template-key chunk is a single contraction tile and the template-query
   columns ride along the search columns in one matmul.
 - x is loaded, cast to bf16 on the DVE, staged to DRAM with the token
   rotation, and read back with XBAR DMA-transpose to give x^T [C, N]
   tiles. Weights are cast f32->bf16 by one SWDGE casting DMA each
   (DRAM->DRAM) so the whole weight preamble runs on the otherwise-idle
   Pool engine instead of serializing ~36 HWDGE DMAs before the first MM.
 - q^T,k^T computed head-major [C, N] (weights stationary); v computed
   token-major [N, C] (x^T stationary) with a ones-column appended per head
   so the attention-value matmul also emits softmax denominators (row 64).
 - Scores are computed key-major S^T = k @ q^T; exp on ScalarE (logits are
   tiny: |s|<~3, so no max subtraction, matching softmax semantics exactly
   up to fp rounding); O^T = (v_aug)^T @ exp(S^T) accumulates over key chunks.
   Heads are processed in pairs at partitions 0:64 / 64:128 with their S
   matmuls interleaved, so consecutive K=64 matmuls land on disjoint PE row
   groups and run concurrently in the array.
 - Normalization: reciprocal of the denominator row (exact DVE op - the
   custom-DVE approx returns garbage on HW via this exec path), gpsimd
   partition broadcast, one fused multiply during the PSUM->SBUF evacuation.
 - proj consumes O^T head-major directly (stationary), emits token-major
   tiles, adds bias broadcast, DMAs straight to the outputs. The template
   block is computed once and written to both outputs.
 - Batches are software-pipelined: batch b+1's staging + qkv emission is
   interleaved with batch b's attend/proj so the Tensor engine has dense
   independent work while ScalarE chews on exp.
"""

import sys

sys.path.insert(0, "/opt/trn_rl_repo")

import numpy as np

B, N, C = 64, 384, 768
H, DH = 12, 64
LT, LS = 128, 256  # template / search token counts
NCORES = 8
BL = B // NCORES  # batches per core
CK = C // 128  # contraction chunks
NT = N // 128  # token tiles
C3 = 3 * C

_CACHE = {}


def _build_program(variant="default"):
    import concourse.tile as tile
    from concourse import bacc, library_config
    import concourse.mybir as mybir

    flags = set(variant.split("+"))

    dt = mybir.dt
    BF, F32, FP8 = dt.bfloat16, dt.float32, dt.float8e4
    Exp = mybir.ActivationFunctionType.Exp
    ActCopy = mybir.ActivationFunctionType.Copy
    DblRow = mybir.MatmulPerfMode.DoubleRow
    # fp8 path: qkv matmuls in e4m3 with DoubleRow (2 K-chunks per
    # instruction, 2 MACs/cell/cycle). Weights are pre-scaled x16 so
    # |w|~0.02 lands in e4m3's normal range; the x16 on q and k cancels
    # via the exp scale (/256), and the x16 on v cancels exactly against
    # a 16.0 ones-column (denominator also x16).
    fp8 = "fp8" in set(variant.split("+"))
    W_SCALE = 16.0
    EXP_SCALE = 0.125 / (W_SCALE * W_SCALE) if fp8 else 0.125
    ONES_VAL = W_SCALE if fp8 else 1.0

    nc = bacc.Bacc("TRN2", target_bir_lowering=False, debug=False)

    x_in = nc.dram_tensor("x", [BL, N, C], F32, kind="ExternalInput")
    xh_in = nc.dram_tensor("x_hsi", [BL, N, C], F32, kind="ExternalInput")
    qkvw_in = nc.dram_tensor("qkv_w", [C3, C], F32, kind="ExternalInput")
    projw_in = nc.dram_tensor("proj_w", [C, C], F32, kind="ExternalInput")
    projb_in = nc.dram_tensor("proj_b", [C], F32, kind="ExternalInput")
    out0 = nc.dram_tensor("out", [BL, N, C], F32, kind="ExternalOutput")
    out1 = nc.dram_tensor("out_hsi", [BL, N, C], F32, kind="ExternalOutput")

    with tile.TileContext(nc) as tc:
        with tc.tile_critical():
            nc.gpsimd.load_library(library_config.attn)

        import contextlib

        stack = contextlib.ExitStack()
        with stack:
            const = stack.enter_context(tc.tile_pool(name="const", bufs=1))
            # stage tiles now hold a whole [N, C] matrix (one load + one
            # cast per matrix); 2 bufs = both streams of a batch in flight.
            stage = stack.enter_context(tc.tile_pool(name="stage", bufs=2))
            dram = stack.enter_context(tc.tile_pool(name="dram", bufs=8, space="DRAM"))
            wdram = stack.enter_context(tc.tile_pool(name="wdram", bufs=1, space="DRAM"))
            # one [128, CK, N] x^T tile per stream-matrix (2 batches deep).
            xtp = stack.enter_context(tc.tile_pool(name="xtp", bufs=4))
            qkp = stack.enter_context(tc.tile_pool(name="qkp", bufs=40 if fp8 else 48))
            vbp = stack.enter_context(tc.tile_pool(name="vbp", bufs=12))
            atp = stack.enter_context(tc.tile_pool(name="atp", bufs=4))
            drp = stack.enter_context(tc.tile_pool(name="drp", bufs=4))
            rbcp = stack.enter_context(tc.tile_pool(name="rbcp", bufs=4))
            xt8p = stack.enter_context(tc.tile_pool(name="xt8p", bufs=12))
            obp = stack.enter_context(tc.tile_pool(name="obp", bufs=12))
            osbp = stack.enter_context(tc.tile_pool(name="osbp", bufs=4))
            psmm = stack.enter_context(tc.tile_pool(name="psmm", bufs=3, space="PSUM"))
            pss = stack.enter_context(tc.tile_pool(name="pss", bufs=2, space="PSUM"))
            pso = stack.enter_context(tc.tile_pool(name="pso", bufs=3, space="PSUM"))

            # ---------------- staging (DRAM, bf16) ----------------
            # cast one [N, C] fp32 matrix to bf16 in DRAM, tokens reordered to
            # [search | template]; read back transposed as x^T SBUF tiles.
            # (SWDGE casting DMAs would be one instruction, but the Q7
            # descriptor generation is charged per 1.5KB row - way slower
            # than this HWDGE + DVE-cast chain.)
            def stage_matrix(src, b):
                xb = dram.tile([N, C], BF, tag="xb")
                # one 3D-AP load for all 3 token tiles (block t on the free
                # axis), one cast, two stores realizing the (t+2)%3 rotation.
                ld = stage.tile([128, NT, C], F32, tag="stageld")
                nc.sync.dma_start(
                    ld[:], src[b].rearrange("(blk p) c -> p blk c", p=128)
                )
                cs = stage.tile([128, NT, C], BF, tag="stagecs")
                if "actstage" in flags:
                    # ScalarE casts collide with the attend-phase Exp
                    # and sim 10us slower overall - kept for reference.
                    nc.scalar.activation(cs[:], ld[:], ActCopy)
                else:
                    nc.vector.tensor_copy(cs[:], ld[:])
                nc.sync.dma_start(
                    xb[0:256, :].rearrange("(blk p) c -> p blk c", p=128),
                    cs[:, 1:3, :],
                )
                nc.sync.dma_start(xb[256:384, :], cs[:, 0, :])
                # one XBAR transpose for all 6 contraction chunks: 3D output
                # [128, 6, N] puts the extra chunks on the free axis
                # ("logically part of the partition dim"). Splitting it in
                # halves for earlier chain starts simmed only 0.6us better -
                # not worth the extra instructions.
                xtt = xtp.tile([128, CK, N], BF, tag="xt")
                nc.sync.dma_start_transpose(xtt[:], xb[:])
                xt = [xtt[:, ci, :] for ci in range(CK)]
                if not fp8:
                    return xt
                xt8 = []
                for c in range(CK // 2):
                    t8 = xt8p.tile([128, 2, N], FP8, tag="xt8")
                    for s in range(2):
                        nc.vector.tensor_copy(t8[:, s, :], xt[2 * c + s])
                    xt8.append(t8)
                return xt8

            # weights: one whole-tensor SWDGE casting DMA each (contiguous -
            # cheap descriptors), then one partition-extended XBAR transpose.
            qkv_wb = wdram.tile([C3, C], BF, tag="qkv_wb")
            proj_wb = wdram.tile([C, C], BF, tag="proj_wb")
            wt = []  # qkv_w^T chunks: wt[ci] = [128 (C rows ci), 2304]
            wt8 = []  # fp8 pair chunks: wt8[c] = [128, 2, 2304], x16
            wpt = []  # proj_w^T chunks: wpt[ci] = [128, 768]

            def _stage_w_hwdge(src, dst, rows):
                for r in range(rows // 128):
                    ld = stage.tile([128, C], F32, tag="stageld")
                    nc.sync.dma_start(ld[:], src[r * 128 : (r + 1) * 128, :])
                    cs = stage.tile([128, C], BF, tag="stagecs")
                    nc.vector.tensor_copy(cs[:], ld[:])
                    nc.sync.dma_start(dst[r * 128 : (r + 1) * 128, :], cs[:])

            def stage_weights_qkv():
                # one contiguous casting DMA (strided column slices would cost
                # one SWDGE descriptor per row), then per-chunk transposes.
                if "hww" in flags:
                    _stage_w_hwdge(qkvw_in, qkv_wb, C3)
                else:
                    nc.gpsimd.dma_start(qkv_wb[:], qkvw_in[:])
                wtt = const.tile([128, CK, C3], BF, tag="wtt")
                nc.sync.dma_start_transpose(wtt[:], qkv_wb[:])
                wt.extend(wtt[:, ci, :] for ci in range(CK))
                if fp8:
                    for c in range(CK // 2):
                        t8 = const.tile([128, 2, C3], FP8, tag=f"wt8_{c}")
                        for s in range(2):
                            nc.vector.tensor_scalar_mul(
                                t8[:, s, :], wt[2 * c + s], W_SCALE
                            )
                        wt8.append(t8)

            def stage_weights_proj():
                if "hww" in flags:
                    _stage_w_hwdge(projw_in, proj_wb, C)
                else:
                    nc.gpsimd.dma_start(proj_wb[:], projw_in[:])
                wptt = const.tile([128, CK, C], BF, tag="wptt")
                nc.sync.dma_start_transpose(wptt[:], proj_wb[:])
                wpt.extend(wptt[:, ci, :] for ci in range(CK))

            # ---------------- per-batch pipeline ----------------
            def qk_chunk(xt, m, hsi):
                """one 128-wide output chunk of q^T/k^T, head-major.
                In fp8 mode xt is the list of [128, 2, N] pair tiles."""
                fm = 256 if (hsi and m < 6) else 384  # hsi q: search only
                ps = psmm.tile([128, 384], F32, tag="mm")
                if fp8:
                    for c in range(CK // 2):
                        nc.tensor.matmul(
                            ps[:, :fm],
                            wt8[c][:, :, m * 128 : (m + 1) * 128],
                            xt[c][:, :, :fm],
                            start=(c == 0),
                            stop=(c == CK // 2 - 1),
                            perf_mode=DblRow,
                        )
                else:
                    for ci in range(CK):
                        nc.tensor.matmul(
                            ps[:, :fm],
                            wt[ci][:, m * 128 : (m + 1) * 128],
                            xt[ci][:, :fm],
                            start=(ci == 0),
                            stop=(ci == CK - 1),
                        )
                t = qkp.tile([128, 384], BF, tag="qk")
                nc.vector.tensor_copy(t[:, :fm], ps[:, :fm])
                return t

            def v_chunk(xt, it, vb_list):
                """one token tile of v, 65-strided heads + ones column (the
                ones carry the fp8 weight descale: O and denom both x16)."""
                t = vbp.tile([128, 12 * 65], BF, tag="vb")
                v3 = t[:].rearrange("p (h e) -> p h e", e=65)
                nc.vector.memset(v3[:, :, 64:65], ONES_VAL)
                psa = psmm.tile([128, 384], F32, tag="mm")
                psb = psmm.tile([128, 384], F32, tag="mm")
                if fp8:
                    for c in range(CK // 2):
                        lhs = xt[c][:, :, it * 128 : (it + 1) * 128]
                        nc.tensor.matmul(
                            psa[:], lhs, wt8[c][:, :, 1536:1920],
                            start=(c == 0), stop=(c == CK // 2 - 1),
                            perf_mode=DblRow,
                        )
                        nc.tensor.matmul(
                            psb[:], lhs, wt8[c][:, :, 1920:2304],
                            start=(c == 0), stop=(c == CK // 2 - 1),
                            perf_mode=DblRow,
                        )
                else:
                    for ci in range(CK):
                        lhs = xt[ci][:, it * 128 : (it + 1) * 128]
                        nc.tensor.matmul(
                            psa[:], lhs, wt[ci][:, 1536:1920],
                            start=(ci == 0), stop=(ci == CK - 1),
                        )
                        nc.tensor.matmul(
                            psb[:], lhs, wt[ci][:, 1920:2304],
                            start=(ci == 0), stop=(ci == CK - 1),
                        )
                nc.vector.tensor_copy(
                    v3[:, 0:6, 0:64], psa[:].rearrange("p (h e) -> p h e", e=64)
                )
                nc.vector.tensor_copy(
                    v3[:, 6:12, 0:64], psb[:].rearrange("p (h e) -> p h e", e=64)
                )
                vb_list.append(t)

            def attend_pair(qk, vb, h0, hsi, obuf):
                """heads h0, h0+1 of one stream: S^T -> exp -> O^T(+denom) ->
                normalize into obuf columns. The two heads sit at partitions
                0:64 / 64:128, so their interleaved S matmuls land on disjoint
                PE row strips and run concurrently in the array."""
                nq = 256 if hsi else 384
                co = 384 if hsi else 0
                heads = (h0, h0 + 1)
                at3s, ops = {}, {}
                for h in heads:
                    at = atp.tile([128, 3 * nq], BF, tag="ath" if hsi else "at")
                    at3s[h] = at[:].rearrange("p (c q) -> p c q", q=nq)
                for ck in (2, 0, 1):
                    fq = nq if ck == 2 else 256
                    sps = {}
                    for h in heads:
                        po = 64 * (h % 2)
                        sp = pss.tile([128, 384], F32, tag="s")
                        nc.tensor.matmul(
                            sp[:, :fq],
                            qk[6 + h // 2][po : po + 64, ck * 128 : (ck + 1) * 128],
                            qk[h // 2][po : po + 64, 0:fq],
                            start=True,
                            stop=True,
                        )
                        sps[h] = sp
                    for h in heads:
                        nc.scalar.activation(
                            at3s[h][:, ck, :fq], sps[h][:, :fq], Exp, scale=EXP_SCALE
                        )
                for i, ck in enumerate((2, 0, 1)):
                    fq = nq if ck == 2 else 256
                    for h in heads:
                        if i == 0:
                            ops[h] = pso.tile([65, 384], F32, tag="o", name=f"op{h}")
                        nc.tensor.matmul(
                            ops[h][:, :fq],
                            vb[ck][:, h * 65 : (h + 1) * 65],
                            at3s[h][:, ck, :fq],
                            start=(i == 0),
                            stop=(i == 2),
                            skip_group_check=True,
                        )
                for h in heads:
                    po, op = 64 * (h % 2), ops[h]
                    if variant == "nonorm":
                        nc.vector.tensor_copy(
                            obuf[h // 2][po : po + 64, co : co + nq], op[0:64, :nq]
                        )
                    else:
                        # NOTE: reciprocal_approx_fast (custom-DVE) returns
                        # garbage on HW via this exec path (its DVE table
                        # never reaches the device) - use the exact op.
                        rec = drp.tile([1, 384], F32, tag="rec")
                        nc.vector.reciprocal(rec[:, :nq], op[64:65, :nq])
                        rbc = rbcp.tile([64, 384], F32, tag="rbc")
                        nc.gpsimd.partition_broadcast(rbc[:, :nq], rec[:, :nq])
                        nc.vector.tensor_mul(
                            obuf[h // 2][po : po + 64, co : co + nq],
                            op[0:64, :nq],
                            rbc[:, :nq],
                        )

            # output column ranges of the 5 proj tiles (internal order):
            # 0: main search 0:128   -> out[b, 128:256]
            # 1: main search 128:256 -> out[b, 256:384]
            # 2: template (shared)   -> out[b, 0:128] and out_hsi[b, 0:128]
            # 3: hsi search 0:128    -> out_hsi[b, 128:256]
            # 4: hsi search 128:256  -> out_hsi[b, 256:384]
            proj_targets = [
                [(out0, 128)],
                [(out0, 256)],
                [(out0, 0), (out1, 0)],
                [(out1, 128)],
                [(out1, 256)],
            ]

            def proj_tile(obuf, b, tt):
                psa = psmm.tile([128, 384], F32, tag="mm")
                psb = psmm.tile([128, 384], F32, tag="mm")
                for ci in range(CK):
                    lhs = obuf[ci][:, tt * 128 : (tt + 1) * 128]
                    nc.tensor.matmul(
                        psa[:], lhs, wpt[ci][:, 0:384],
                        start=(ci == 0), stop=(ci == CK - 1),
                    )
                    nc.tensor.matmul(
                        psb[:], lhs, wpt[ci][:, 384:768],
                        start=(ci == 0), stop=(ci == CK - 1),
                    )
                ob = osbp.tile([128, C], F32, tag="outsb")
                nc.vector.tensor_add(ob[:, 0:384], psa[:], bias_bc[:, 0:384])
                nc.vector.tensor_add(ob[:, 384:768], psb[:], bias_bc[:, 384:768])
                for dst, row in proj_targets[tt]:
                    nc.sync.dma_start(dst[b, row : row + 128, :], ob[:])

            # ---------------- emission ----------------
            # per-batch unit generators; interleaving batch b's attend units
            # with batch b+1's qkv units keeps PE dense while ACT runs exp.
            state = {}

            def qkv_units(b):
                """yields thunks; populates state[b] = (qk_m, vb_m, qk_h, vb_h).

                v tiles first, then qk chunks in head-pair order (q chunk p,
                then k chunk 6+p) so when the tail of the LAST batch's qkv is
                interleaved into its own attend stream, each attend pair's
                operands land just before the pair needs them. qk lists are
                indexed by m, so pre-size them and assign by slot."""
                qk_m, vb_m = [None] * 12, []
                qk_h, vb_h = [None] * 12, []
                state[b] = (qk_m, vb_m, qk_h, vb_h)
                xt_m = stage_matrix(x_in, b)
                xt_h = stage_matrix(xh_in, b)
                for it in range(NT):
                    yield lambda it=it: v_chunk(xt_m, it, vb_m)
                    yield lambda it=it: v_chunk(xt_h, it, vb_h)
                for p in range(6):
                    for m in (p, 6 + p):
                        yield lambda m=m: qk_m.__setitem__(
                            m, qk_chunk(xt_m, m, False)
                        )
                        yield lambda m=m: qk_h.__setitem__(
                            m, qk_chunk(xt_h, m, True)
                        )

            def attend_units(b):
                qk_m, vb_m, qk_h, vb_h = state[b]
                obuf = [
                    obp.tile([128, 640], BF, tag="obuf", name=f"obuf_{b}_{j}")
                    for j in range(CK)
                ]
                for hp in range(0, H, 2):
                    yield lambda hp=hp: attend_pair(qk_m, vb_m, hp, False, obuf)
                    yield lambda hp=hp: attend_pair(qk_h, vb_h, hp, True, obuf)
                for tt in range(5):
                    yield lambda tt=tt: proj_tile(obuf, b, tt)
                del state[b]

            def drain(g):
                for f in g:
                    f()

            # preamble: batch-0 input staging + qkv weights first, proj
            # weights late (first needed ~one batch in).
            gen_q = qkv_units(0)
            stage_weights_qkv()
            bias1 = const.tile([1, C], F32, tag="bias1")
            nc.sync.dma_start(bias1[:], projb_in[:].unsqueeze(0))
            bias_bc = const.tile([128, C], F32, tag="bias_bc")
            nc.gpsimd.partition_broadcast(bias_bc[:], bias1[:])
            drain(gen_q)
            stage_weights_proj()

            # the last batch has no successor to fill PE gaps during its
            # attend phase, so only the first half of its qkv units is
            # interleaved with attend(BL-2); the rest carries over into
            # attend(BL-1)'s stream as Tensor-engine filler.
            import itertools

            carry = iter(())
            for b in range(BL):
                gen_a = attend_units(b)
                if b + 1 < BL:
                    gen_q = qkv_units(b + 1)
                    # 20/10 split: the 10 carried qk units stay ahead of the
                    # attend pairs that consume them under 1:1 interleave.
                    if b + 1 == BL - 1:
                        first, carry_next = (
                            itertools.islice(gen_q, 20),
                            gen_q,
                        )
                    else:
                        first, carry_next = gen_q, iter(())
                else:
                    first, carry_next = iter(()), iter(())
                gen_q = itertools.chain(carry, first)
                while True:
                    fa = next(gen_a, None)
                    if fa is not None:
                        fa()
                    fq = next(gen_q, None)
                    if fq is not None:
                        fq()
                    if fa is None and fq is None:
                        break
                carry = carry_next

    nc.compile()
    return nc


def _get_program(variant="default"):
    if variant not in _CACHE:
        _CACHE[variant] = _build_program(variant)
    return _CACHE[variant]


def kernel(x, x_hsi, qkv_w, proj_w, proj_b, t_h=8, t_w=8, s_h=16, s_w=16,
           num_heads=12, _variant="default", **_ignored):
    from concourse.bass_utils import run_bass_kernel_spmd

    nc = _get_program(_variant)
    x = np.asarray(x, dtype=np.float32)
    x_hsi = np.asarray(x_hsi, dtype=np.float32)
    qkv_w = np.asarray(qkv_w, dtype=np.float32)
    proj_w = np.asarray(proj_w, dtype=np.float32)
    proj_b = np.asarray(proj_b, dtype=np.float32)

    core_ids = list(range(NCORES))
    in_maps = [
        {
            "x": x[c * BL : (c + 1) * BL],
            "x_hsi": x_hsi[c * BL : (c + 1) * BL],
            "qkv_w": qkv_w,
            "proj_w": proj_w,
            "proj_b": proj_b,
        }
        for c in core_ids
    ]
    res = run_bass_kernel_spmd(nc, in_maps, core_ids)
    out = np.concatenate([res.results[c]["out"] for c in core_ids], axis=0)
    out_hsi = np.concatenate([res.results[c]["out_hsi"] for c in core_ids], axis=0)
    return out, out_hsi

